# revision 1
# baseline (speedup 1.0000x reference)
"""Trainium2 Bass kernel for nn_Cluster_Level_GCN (gnn_message_passing).

kernel(**inputs) takes the FULL inputs (as in reference.setup_inputs()) and
returns the full [B, K, 2] softmax output, distributing work over 8
NeuronCores:
  - cluster-mean segment-sum sharded by cluster (256 clusters/core), rows
    routed to the owning core via a host-side label sort (index-only work)
  - AllGather of the per-core cluster means into a replicated table
  - data-parallel GCN over the batch (8 queries/core)
"""

import sys

sys.path.insert(0, "/opt/trn_rl_repo")

import numpy as np

from concourse import bass, mybir, tile
from concourse.bass import IndirectOffsetOnAxis
from concourse.bass_utils import run_bass_kernel_spmd
from concourse.masks import make_identity

# Problem shapes (hardcoded per contract)
B, K, N, D = 64, 64, 32768, 2048
NC, NHID, TOPK = 2048, 512, 5
NCORES = 8
SAMP = B // NCORES          # samples per core
CLC = NC // NCORES          # clusters per core
F32 = mybir.dt.float32
F32R = mybir.dt.float32r
I32 = mybir.dt.int32
AL = mybir.AluOpType

_legal_n = [0]


def _legalize_multiwait(nc):
    """This container's walrus rejects instructions with >1 sync waits
    ("Too many sync wait commands").  Hoist extra waits onto standalone
    single-wait InstEventSemaphore instructions placed just before."""
    for f in nc.m.functions:
        for bb in f.blocks:
            insts = bb.instructions
            if not any(
                i.sync_info is not None and len(i.sync_info.on_wait) > 1
                for i in insts
            ):
                continue
            new = []
            for ins in insts:
                si = ins.sync_info
                if si is not None and len(si.on_wait) > 1:
                    for w in si.on_wait[:-1]:
                        _legal_n[0] += 1
                        new.append(
                            mybir.InstEventSemaphore(
                                name=f"I-lgl-{_legal_n[0]}",
                                ins=[],
                                outs=[],
                                engine=ins.engine,
                                sync_info=mybir.SyncInfo(
                                    on_wait=[w], on_update=[]
                                ),
                            )
                        )
                    ins.sync_info = mybir.SyncInfo(
                        on_wait=[si.on_wait[-1]], on_update=si.on_update
                    )
                new.append(ins)
            bb.instructions = new
    return nc


def build_kernel(chunks_h):
    """Build the SPMD Bass program.  chunks_h = number of 128-row chunks per
    cluster-half (two halves of 128 clusters per core)."""
    nc = bass.Bass(trn_type="TRN2", target_bir_lowering=False, debug=False,
                   num_devices=NCORES)

    RPH = chunks_h * 128  # rows per half (padded)
    NCH = 16  # d-chunks of 128

    # ---- external inputs (per-core values supplied via in_maps)
    featrows = nc.dram_tensor("featrows", [2 * RPH, D], F32, kind="ExternalInput")
    onehot = nc.dram_tensor("onehot", [2 * chunks_h, 128, 128], F32, kind="ExternalInput")
    cntm1 = nc.dram_tensor("cntm1", [2 * 128, 1], F32, kind="ExternalInput")
    qfeat = nc.dram_tensor("qfeat", [SAMP, D], F32, kind="ExternalInput")
    clu_idx = nc.dram_tensor("clu_idx", [SAMP * K, 1], I32, kind="ExternalInput")
    keep_o = nc.dram_tensor("keep_o", [K, SAMP * K], F32, kind="ExternalInput")
    # conv_w host-relaid as [o*2+half][p][ci*128+f] so each load is one
    # plain contiguous [128, 2048] DMA (f32r: PE rounds in-datapath)
    cwdev = nc.dram_tensor("cwdev", [8, 128, NCH * 128], F32R, kind="ExternalInput")
    w1 = nc.dram_tensor("w1", [NHID, NHID], F32, kind="ExternalInput")
    w2d = nc.dram_tensor("w2d", [NHID, 2], F32, kind="ExternalInput")
    cb_d = nc.dram_tensor("cb_d", [128, 4], F32, kind="ExternalInput")   # conv_b  [p, o]
    b1_d = nc.dram_tensor("b1_d", [128, 4], F32, kind="ExternalInput")   # b1
    a_d = nc.dram_tensor("a_d", [128, 4], F32, kind="ExternalInput")     # prelu_a
    b2_d = nc.dram_tensor("b2_d", [1, 2], F32, kind="ExternalInput")     # b2
    out_d = nc.dram_tensor("out", [1, SAMP * K * 2], F32, kind="ExternalOutput")

    with tile.TileContext(nc) as tc:
        with (
            tc.tile_pool(name="consts", bufs=1) as cpool,
            tc.tile_pool(name="dram", bufs=1, space="DRAM") as dpool,
        ):
            # internal DRAM (pool tiles so Tile tracks collective/gather deps)
            bounce = dpool.tile([CLC, D], F32, name="bounce")
            table = dpool.tile([NC, D], F32, addr_space="Shared",
                               name="table")
            # ---------- long-lived constants
            ident = cpool.tile([128, 128], F32)
            make_identity(nc, ident[:])
            ones1 = cpool.tile([1, 128], F32)
            nc.vector.memset(ones1[:], 1.0)

            # ================= PHASE A =================
            with (
                tc.tile_pool(name="feat", bufs=3) as fpool,
                tc.tile_pool(name="oh", bufs=3) as opool,
                tc.tile_pool(name="psA", bufs=1, space="PSUM") as psA,
                tc.tile_pool(name="sA", bufs=2) as sA,
            ):
                cnt_t = sA.tile([128, 2], F32, tag="cnt")
                nc.sync.dma_start(out=cnt_t[:, 0:1], in_=cntm1[0:128, :])
                nc.sync.dma_start(out=cnt_t[:, 1:2], in_=cntm1[128:256, :])
                cnt_r = sA.tile([128, 2], F32, tag="cntr")
                nc.vector.reciprocal(out=cnt_r[:], in_=cnt_t[:])
                ps_half = [psA.tile([128, D], F32, tag=f"psA{h}", name=f"psA{h}") for h in range(2)]
                for h in range(2):
                    for c in range(chunks_h):
                        ft = fpool.tile([128, D], F32)
                        nc.sync.dma_start(
                            out=ft[:],
                            in_=featrows[(h * chunks_h + c) * 128:
                                         (h * chunks_h + c + 1) * 128, :])
                        oh = opool.tile([128, 128], F32)
                        nc.sync.dma_start(out=oh[:], in_=onehot[h * chunks_h + c, :, :])
                        for seg in range(4):
                            nc.tensor.matmul(
                                ps_half[h][:, seg * 512:(seg + 1) * 512],
                                lhsT=oh[:],
                                rhs=ft[:, seg * 512:(seg + 1) * 512],
                                start=(c == 0), stop=(c == chunks_h - 1))
                    mean_t = sA.tile([128, D], F32, tag="mean")
                    nc.vector.tensor_scalar(
                        out=mean_t[:], in0=ps_half[h][:],
                        scalar1=cnt_r[:, h:h + 1], scalar2=None,
                        op0=AL.mult)
                    nc.sync.dma_start(out=bounce[h * 128:(h + 1) * 128, :],
                                      in_=mean_t[:])

                nc.gpsimd.collective_compute(
                    "AllGather", AL.bypass,
                    replica_groups=[list(range(NCORES))],
                    ins=[bounce[:].opt()],
                    outs=[table[0:NC, :].opt()],
                )

            # ================= PHASE B + C =================
            with (
                tc.tile_pool(name="cf", bufs=1) as cfpool,
                tc.tile_pool(name="xT", bufs=1) as xpool,
                tc.tile_pool(name="agg", bufs=1) as apool,
                tc.tile_pool(name="sB", bufs=2) as sB,
                tc.tile_pool(name="sBig", bufs=2) as sBig,
                tc.tile_pool(name="psS", bufs=4, space="PSUM") as psS,
                tc.tile_pool(name="psL", bufs=2, space="PSUM") as psL,
                tc.tile_pool(name="cw", bufs=4) as cwpool,
                tc.tile_pool(name="hh", bufs=1) as hpool,
            ):
                R = SAMP * K  # 512 rows on this core

                keep_t = sBig.tile([K, R], F32, tag="keep", bufs=1)
                nc.sync.dma_start(out=keep_t[:], in_=keep_o[:])

                cf_p = [cfpool.tile([128, D], F32, tag=f"cf{p}", name=f"cf{p}") for p in range(4)]
                xT = [xpool.tile([128, R], F32, tag=f"xT{c}", name=f"xT{c}") for c in range(NCH)]
                A5all = sBig.tile([64, R], F32, tag="A5all", bufs=1)
                # adjMD: sample s occupies partitions [(s%2)*64, +64) and
                # free [s*K, (s+1)*K); the other 64 partitions stay ZERO so a
                # pair-batched full-128-contract matmul computes aggT.
                adjMD = sBig.tile([128, R], F32, tag="adjMD", bufs=1)
                nc.vector.memset(adjMD[:], 0.0)
                nsq8 = sB.tile([64, SAMP], F32, tag="nsq8", bufs=1)
                negs = sB.tile([1, R], F32, tag="negs", bufs=1)
                invnrow = sB.tile([1, R], F32, tag="invnrow", bufs=1)

                for p in range(4):
                    it = sB.tile([128, 1], I32, tag="gidx")
                    nc.sync.dma_start(out=it[:], in_=clu_idx[p * 128:(p + 1) * 128, :])
                    nc.gpsimd.indirect_dma_start(
                        out=cf_p[p][:], out_offset=None, in_=table[:],
                        in_offset=IndirectOffsetOnAxis(ap=it[:, :1], axis=0))
                    # row 0 of each sample is the raw query feature row
                    for s in (2 * p, 2 * p + 1):
                        nc.sync.dma_start(
                            out=cf_p[p][(s % 2) * 64:(s % 2) * 64 + 1, :],
                            in_=qfeat[s:s + 1, :])
                    for c in range(NCH):
                        tp = psS.tile([128, 128], F32, tag="s")
                        nc.tensor.transpose(out=tp[:],
                                            in_=cf_p[p][:, c * 128:(c + 1) * 128],
                                            identity=ident[:])
                        nc.vector.tensor_copy(out=xT[c][:, p * 128:(p + 1) * 128],
                                              in_=tp[:])

                for s in range(SAMP):
                    sl = slice(s * K, (s + 1) * K)
                    A_ps = psS.tile([64, 64], F32, tag="s")
                    for c in range(NCH):
                        nc.tensor.matmul(A_ps[:], lhsT=xT[c][:, sl],
                                         rhs=xT[c][:, sl],
                                         start=(c == 0), stop=(c == NCH - 1))
                    dscr = sB.tile([64, 64], F32, tag="dscr")
                    nc.vector.scalar_tensor_tensor(
                        out=dscr[:], in0=A_ps[:], scalar=0.0,
                        in1=ident[:64, :64],
                        op0=AL.add, op1=AL.mult,
                        accum_out=nsq8[:, s:s + 1])
                    nc.vector.tensor_scalar(
                        out=A5all[:, sl], in0=A_ps[:], scalar1=0.2, scalar2=None,
                        op0=AL.mult)

                nsqrt8 = sB.tile([64, SAMP], F32, tag="nsqrt8", bufs=1)
                invn8 = sB.tile([64, SAMP], F32, tag="invn8", bufs=1)
                negn8 = sB.tile([128, SAMP], F32, tag="negn8", bufs=1)
                nc.scalar.activation(out=nsqrt8[:], in_=nsq8[:],
                                     func=mybir.ActivationFunctionType.Sqrt)
                nc.vector.reciprocal(out=invn8[:], in_=nsqrt8[:])
                nc.vector.tensor_scalar_mul(negn8[0:64, :], nsqrt8[:], -1.0)
                nc.vector.tensor_copy(out=negn8[64:128, :], in_=negn8[0:64, :])

                ball_ps = psL.tile([128, R], F32, tag="big")
                for s in range(SAMP):
                    tp1 = psS.tile([1, 64], F32, tag="s")
                    nc.tensor.transpose(out=tp1[:], in_=invn8[:, s:s + 1],
                                        identity=ident[:64, :64])
                    nc.vector.tensor_copy(out=invnrow[:, s * K:(s + 1) * K],
                                          in_=tp1[:])
                for s in range(SAMP):
                    nc.tensor.matmul(ball_ps[:, s * K:(s + 1) * K], lhsT=ones1[:],
                                     rhs=invnrow[:, s * K:(s + 1) * K],
                                     start=True, stop=True)
                ball_sb = sBig.tile([128, R], F32, tag="ballsb", bufs=1)
                nc.vector.tensor_copy(out=ball_sb[:], in_=ball_ps[:])

                for s in range(SAMP):
                    sl = slice(s * K, (s + 1) * K)
                    bs = (s % 2) * 64
                    t8 = sB.tile([64, 8], F32, tag="t8")
                    nc.vector.max(out=t8[:], in_=A5all[:, sl])
                    m0 = sB.tile([64, 64], F32, tag="m0")
                    nc.vector.tensor_scalar(
                        out=m0[:], in0=A5all[:, sl],
                        scalar1=t8[:, TOPK - 1:TOPK],
                        scalar2=None, op0=AL.is_ge)
                    m0T_ps = psS.tile([64, 64], F32, tag="s")
                    nc.tensor.transpose(out=m0T_ps[:], in_=m0[:],
                                        identity=ident[:64, :64])
                    msym = sB.tile([64, 64], F32, tag="msym")
                    nc.vector.tensor_tensor(out=msym[:], in0=m0[:], in1=m0T_ps[:],
                                            op=AL.mult)
                    nc.vector.tensor_tensor(out=msym[:], in0=msym[:],
                                            in1=keep_t[:, sl], op=AL.mult)
                    nc.vector.scalar_tensor_tensor(
                        out=adjMD[bs:bs + 64, sl], in0=A5all[:, sl],
                        scalar=invn8[:, s:s + 1],
                        in1=msym[:], op0=AL.mult, op1=AL.mult)
                    negs_ps = psS.tile([1, 64], F32, tag="s")
                    nc.tensor.matmul(negs_ps[:], lhsT=negn8[bs:bs + 64, s:s + 1],
                                     rhs=adjMD[bs:bs + 64, sl],
                                     start=True, stop=True)
                    nc.vector.tensor_copy(out=negs[:, sl], in_=negs_ps[:])
                    # fold the query-centering correction into row 0 of the
                    # sample's adjacency block:
                    #   adjMD[row0, i] = (adj[0, i] - rowsum_i) * invn_0
                    r0 = sB.tile([1, 64], F32, tag="r0")
                    nc.vector.tensor_tensor(out=r0[:], in0=A5all[0:1, sl],
                                            in1=msym[0:1, :], op=AL.mult)
                    nc.vector.tensor_tensor(out=r0[:], in0=r0[:],
                                            in1=negs[:, sl], op=AL.add)
                    nc.vector.tensor_scalar_mul(
                        adjMD[bs:bs + 1, sl], r0[:], invn8[0:1, s:s + 1])

                xr = [xpool.tile([128, R], F32R, tag=f"xr{c}", name=f"xr{c}")
                      for c in range(NCH)]
                for c in range(NCH):
                    nc.vector.tensor_tensor(out=xr[c][:], in0=xT[c][:],
                                            in1=ball_sb[:], op=AL.mult)

                aggT = [apool.tile([128, R], F32R, tag=f"ag{c}", name=f"ag{c}") for c in range(NCH)]
                for c in range(NCH):
                    for p in range(4):
                        pl2 = slice(p * 128, (p + 1) * 128)
                        ag_ps = psS.tile([128, 128], F32, tag="s")
                        nc.tensor.matmul(
                            ag_ps[:],
                            lhsT=cf_p[p][:, c * 128:(c + 1) * 128],
                            rhs=adjMD[:, pl2], start=True, stop=True)
                        nc.vector.tensor_copy(out=aggT[c][:, pl2], in_=ag_ps[:])

                # ---------- GCN tail
                cb_t = sB.tile([128, 4], F32, tag="cb", bufs=1)
                nc.sync.dma_start(out=cb_t[:], in_=cb_d[:])
                b1_t = sB.tile([128, 4], F32, tag="b1", bufs=1)
                nc.sync.dma_start(out=b1_t[:], in_=b1_d[:])
                a_t = sB.tile([128, 4], F32, tag="a", bufs=1)
                nc.sync.dma_start(out=a_t[:], in_=a_d[:])
                b2_t = sB.tile([1, 2], F32, tag="b2", bufs=1)
                nc.sync.dma_start(out=b2_t[:], in_=b2_d[:])
                w1_t = [sBig.tile([128, NHID], F32R, tag=f"w1_{c}", bufs=1, name=f"w1t{c}")
                        for c in range(4)]
                for c in range(4):
                    nc.gpsimd.dma_start(out=w1_t[c][:], in_=w1[c * 128:(c + 1) * 128, :])
                w2_t = sB.tile([128, 8], F32R, tag="w2", bufs=1)
                for c in range(4):
                    nc.gpsimd.dma_start(out=w2_t[:, 2 * c:2 * c + 2],
                                        in_=w2d[c * 128:(c + 1) * 128, :])

                hT = [hpool.tile([128, R], F32R, tag=f"hT{o}", name=f"hT{o}") for o in range(4)]
                for o in range(4):
                    ph = psL.tile([128, R], F32, tag="big")
                    pwx = psS.tile([128, SAMP], F32, tag="s")
                    for half in range(2):
                        cwb = cwpool.tile([128, NCH * 128], F32R, tag="cw",
                                          bufs=2, name=f"cwb{o}_{half}")
                        nc.sync.dma_start(out=cwb[:], in_=cwdev[o * 2 + half, :, :])
                        for ci in range(NCH):
                            c = half * NCH + ci
                            lhs_c = cwb[:, ci * 128:(ci + 1) * 128]
                            rhs = xr[c][:] if c < NCH else aggT[c - NCH][:]
                            nc.tensor.matmul(ph[:], lhsT=lhs_c, rhs=rhs,
                                             start=(c == 0),
                                             stop=(c == 2 * NCH - 1))
                            if c < NCH:
                                nc.tensor.matmul(
                                    pwx[:], lhsT=lhs_c,
                                    rhs=xr[c][:].rearrange(
                                        "p (s k) -> p s k", k=K)[:, :, 0],
                                    start=(c == 0), stop=(c == NCH - 1))
                    bf = sB.tile([128, SAMP], F32, tag="bf")
                    nc.vector.tensor_tensor(
                        out=bf[:], in0=cb_t[:, o:o + 1].to_broadcast([128, SAMP]),
                        in1=pwx[:], op=AL.subtract)
                    for s in range(SAMP):
                        sl = slice(s * K, (s + 1) * K)
                        nc.vector.tensor_scalar(
                            out=hT[o][:, sl], in0=ph[:, sl],
                            scalar1=bf[:, s:s + 1],
                            scalar2=0.0, op0=AL.add, op1=AL.max)

                h1T = [hpool.tile([128, R], F32R, tag=f"h1T{o}", name=f"h1T{o}") for o in range(4)]
                for o in range(4):
                    ph1 = psL.tile([128, R], F32, tag="big")
                    for c in range(4):
                        nc.tensor.matmul(
                            ph1[:],
                            lhsT=w1_t[c][:, o * 128:(o + 1) * 128],
                            rhs=hT[c][:],
                            start=(c == 0), stop=(c == 3))
                    pre = sBig.tile([128, R], F32, tag="pre")
                    nc.vector.tensor_scalar(out=pre[:], in0=ph1[:],
                                            scalar1=b1_t[:, o:o + 1], scalar2=None,
                                            op0=AL.add)
                    pos = sBig.tile([128, R], F32, tag="pos")
                    nc.vector.tensor_scalar(out=pos[:], in0=pre[:], scalar1=0.0,
                                            scalar2=None, op0=AL.max)
                    nc.vector.tensor_scalar(out=pre[:], in0=pre[:], scalar1=0.0,
                                            scalar2=None, op0=AL.min)
                    nc.vector.scalar_tensor_tensor(
                        out=h1T[o][:], in0=pre[:], scalar=a_t[:, o:o + 1],
                        in1=pos[:], op0=AL.mult, op1=AL.add)

                pl0 = psS.tile([1, R], F32, tag="s")
                pl1 = psS.tile([1, R], F32, tag="s")
                for c in range(4):
                    nc.tensor.matmul(pl0[:], lhsT=w2_t[:, 2 * c:2 * c + 1],
                                     rhs=h1T[c][:],
                                     start=(c == 0), stop=(c == 3))
                    nc.tensor.matmul(pl1[:], lhsT=w2_t[:, 2 * c + 1:2 * c + 2],
                                     rhs=h1T[c][:],
                                     start=(c == 0), stop=(c == 3))
                lg0 = sB.tile([1, R], F32, tag="lg0", bufs=1)
                lg1 = sB.tile([1, R], F32, tag="lg1", bufs=1)
                nc.vector.tensor_scalar(out=lg0[:], in0=pl0[:],
                                        scalar1=b2_t[:, 0:1], scalar2=None,
                                        op0=AL.add)
                nc.vector.tensor_scalar(out=lg1[:], in0=pl1[:],
                                        scalar1=b2_t[:, 1:2], scalar2=None,
                                        op0=AL.add)
                dl = sB.tile([1, R], F32, tag="dl", bufs=1)
                nc.vector.tensor_tensor(out=dl[:], in0=lg0[:], in1=lg1[:],
                                        op=AL.subtract)
                p0 = sB.tile([1, R], F32, tag="p0", bufs=1)
                nc.scalar.activation(out=p0[:], in_=dl[:],
                                     func=mybir.ActivationFunctionType.Sigmoid)
                outt = sB.tile([1, 2 * R], F32, tag="outt", bufs=1)
                o3 = outt[:].rearrange("p (k c) -> p k c", c=2)
                nc.vector.tensor_copy(out=o3[:, :, 0:1], in_=p0[:, :, None])
                nc.vector.tensor_scalar(out=o3[:, :, 1:2], in0=p0[:, :, None],
                                        scalar1=-1.0, scalar2=1.0,
                                        op0=AL.mult, op1=AL.add)
                nc.sync.dma_start(out=out_d[:], in_=outt[:])

    _legalize_multiwait(nc)
    return nc


# ---------------------------------------------------------------------------
# host side
# ---------------------------------------------------------------------------

def _preprocess(indexes, features, labels, ori_knn_neighbor,
                conv_w, conv_b, w1, b1, prelu_a, w2, b2):
    indexes = np.asarray(indexes).astype(np.int64)
    labels = np.asarray(labels).astype(np.int64)
    nbr = np.asarray(ori_knn_neighbor).astype(np.int64)
    features = np.ascontiguousarray(np.asarray(features, dtype=np.float32))

    counts = np.bincount(labels, minlength=NC)
    cntm1 = np.maximum(counts, 1).astype(np.float32)

    order = np.argsort(labels, kind="stable")
    slab = labels[order]
    bounds = np.searchsorted(slab, np.arange(0, NC + 1, 128))
    half_rows = [order[bounds[i]:bounds[i + 1]] for i in range(16)]
    chunks_h = max(1, max((len(r) + 127) // 128 for r in half_rows))
    RPH = chunks_h * 128

    clu_lab = labels[nbr]                        # [B, K]
    keep = np.ones((B, K), dtype=np.float32)
    for b in range(B):
        seen = set()
        for k in range(K):
            l = int(clu_lab[b, k])
            if l in seen:
                keep[b, k] = 0.0
            else:
                seen.add(l)

    per_core = []
    for core in range(NCORES):
        fr = np.zeros((2 * RPH, D), dtype=np.float32)
        oh = np.zeros((2 * chunks_h, 128, 128), dtype=np.float32)
        oh2 = oh.reshape(2 * chunks_h * 128, 128)
        for h in range(2):
            rows = half_rows[2 * core + h]
            nr = len(rows)
            fr[h * RPH:h * RPH + nr] = features[rows]
            lab_loc = (labels[rows] - (core * CLC + h * 128)).astype(np.int64)
            oh2[h * RPH + np.arange(nr), lab_loc] = 1.0

        smp = np.arange(core * SAMP, (core + 1) * SAMP)
        cidx = clu_lab[smp].astype(np.int32).copy()      # [SAMP, K]
        cidx[:, 0] = 0  # overwritten on-device by the query row
        qf = features[indexes[smp]]

        ko = np.empty((K, SAMP * K), dtype=np.float32)
        for si, b in enumerate(smp):
            ko[:, si * K:(si + 1) * K] = np.outer(keep[b], keep[b])

        per_core.append(dict(
            featrows=fr,
            onehot=oh,
            cntm1=cntm1[core * CLC:(core + 1) * CLC].reshape(256, 1),
            qfeat=np.ascontiguousarray(qf),
            clu_idx=cidx.reshape(SAMP * K, 1),
            keep_o=ko,
            cwdev=np.ascontiguousarray(
                np.asarray(conv_w, dtype=np.float32)
                .reshape(2, 16, 128, 4, 128)
                .transpose(3, 0, 2, 1, 4)
                .reshape(8, 128, 16 * 128)),
            w1=np.ascontiguousarray(np.asarray(w1, dtype=np.float32)),
            w2d=np.ascontiguousarray(np.asarray(w2, dtype=np.float32)),
            cb_d=np.asarray(conv_b, dtype=np.float32).reshape(4, 128).T.copy(),
            b1_d=np.asarray(b1, dtype=np.float32).reshape(4, 128).T.copy(),
            a_d=np.asarray(prelu_a, dtype=np.float32).reshape(4, 128).T.copy(),
            b2_d=np.asarray(b2, dtype=np.float32).reshape(1, 2),
        ))
    return per_core, chunks_h


_cache = {}


def kernel(indexes, features, labels, ori_knn_neighbor,
           conv_w, conv_b, w1, b1, prelu_a, w2, b2):
    in_maps, chunks_h = _preprocess(
        indexes, features, labels, ori_knn_neighbor,
        conv_w, conv_b, w1, b1, prelu_a, w2, b2)
    if chunks_h not in _cache:
        _cache[chunks_h] = build_kernel(chunks_h)
    nc = _cache[chunks_h]
    res = run_bass_kernel_spmd(nc, in_maps, core_ids=list(range(NCORES)))
    out = np.concatenate(
        [res.results[c]["out"].reshape(SAMP, K, 2) for c in range(NCORES)], axis=0)
    return out

